# revision 42
# baseline (speedup 1.0000x reference)
"""Trainium2 Bass kernel for a bidirectional RNN language model.

Model: emb = embedding[input_batch]; two 16-wide tanh RNN scans (L->R and
R->L) over 128 steps; logits = [hLR, hRL_flipped] @ W_ho.T + b_ho;
log_softmax over vocab 32000. Output [128, 32, 32000] f32 (~524 MB).

Distribution: data-parallel over the 4096 flat (seq*batch) positions,
512 per core (core c owns seq slots [16c, 16c+16)). The tiny recurrence
is replicated on every core; each core computes logits + log_softmax for
its own positions only, selected at runtime via partition_id() dynamic
slices (no collectives -- softmax reduces over vocab, core-local).

Recurrence: ONE matmul + ONE tanh per step. The step state+input lives
in a single [113, 4096] bf16 SBUF tile R (one 32-col block per step):
  rows   0:16  hLR state   (written by tanh, partition base 0)
  rows  16:32  zeros       (pad so tanh output is one [48,32] op)
  rows  32:48  hRL state   (partition base 32)
  rows  48:80  emb[k]      (LR input, DMA-filled)
  rows  80:112 emb[127-k]  (RL input, DMA-filled)
  row   112    ones        (bias row, DMA-filled)
The merged weight Wall [113, 48] has zero blocks so a single
lhsT=Wall matmul computes both chains' next states into one [48,32]
PSUM tile; one tanh writes them back to R block k+1. Per-step latency
is sem/drain-bound (~660 ns in the cost model), so the recurrence is
~88 us regardless of matmul width; bf16 keeps DMA and copies cheap.

Output stage, two matmul passes per 128-position tile, interleaved at
1024-col chunk granularity across tiles (pass1 of tile t with pass2 of
tile t-1) so ACT / DVE / PE / DMA all run concurrently:
  pass1: logits matmul (lhsT = [65,128] bf16 stage slice) -> PSUM; ACT
  exp in-place with accum_out giving sum(exp) per chunk; -ln(sum) via a
  DVE bit-trick polynomial (no ACT table switch). Tile 0 additionally
  stages its logits to SBUF fp16 via DVE (idle during the fill), so its
  pass2 needs no matmul and runs as 2x-mode SBUF adds.
  pass2: matmul again + evacuate PSUM adding -ln(S) (DVE
  tensor_scalar_add; the post-pass1 tail splits whole output stripes
  between DVE and ACT Identity-with-bias, with separate PSUM pools per
  engine). DMA out in fp16 (halves the store-bandwidth floor; adds
  ~3e-4 rel err vs the 2e-2 gate). Host casts fp16->f32.

log_softmax max-subtraction is dropped: logits are bounded (~|7|), so
f32 exp cannot overflow; out = logits - ln(sum exp(logits)).

Host-side work is limited to layout transforms (transposes, zero-pad,
bias-row augmentation), the embedding row gather, and the final fp16->
f32 cast; all arithmetic (projections, recurrences, logits matmul,
softmax) runs on the NeuronCores.
"""

import os

import numpy as np
import ml_dtypes

SEQ, B, VOCAB = 128, 32, 32000
EMB, HID = 32, 16
NCORES = 8
POS = SEQ * B                 # 4096 flat positions, f = s*B + b
PPC = POS // NCORES           # 512 positions per core
PTILES = PPC // 128           # 4 position tiles of 128 per core
RROWS = 113                   # R tile rows (see module docstring)
KDIM = 65                     # stage/who contraction rows
GW = 1024                     # exp group / pass-2 chunk width (2 PSUM banks)
GROUPS = [(g * GW, GW) for g in range(31)] + [(31 * GW, VOCAB - 31 * GW)]
SW = 2048                     # output DMA stripe width (0.5 MB fp16)
STRIPES = [(s * SW, min(SW, VOCAB - s * SW)) for s in range((VOCAB + SW - 1) // SW)]
ACT_EVERY = 999               # 1 of every ACT_EVERY pass-2 chunks evacuates on ACT


def _mm_splits(j0, w):
    out = []
    j = j0
    while j < j0 + w:
        jw = min(512, j0 + w - j)
        out.append((j, jw))
        j += jw
    return out


_CACHE = {}


def _build(loop=1):
    if ("nc", loop) in _CACHE:
        return _CACHE[("nc", loop)]

    import concourse.bass as bass
    import concourse.tile as tile
    from concourse import bacc, mybir

    f32 = mybir.dt.float32
    f16 = mybir.dt.float16
    bf16 = mybir.dt.bfloat16
    AF = mybir.ActivationFunctionType

    nc = bacc.Bacc(
        "TRN2",
        target_bir_lowering=False,
        debug=False,
        num_devices=NCORES,
    )

    # Name suffix per loop variant: the jitted HLO is otherwise identical
    # across variants (the bass module travels out-of-band), so XLA's
    # compile cache would hand the loop build the non-loop NEFF.
    sfx = f"_L{loop}" if loop > 1 else ""
    d_rstat = nc.dram_tensor(f"rstat{sfx}", [RROWS, POS], bf16, kind="ExternalInput").ap()
    d_wall = nc.dram_tensor(f"wall{sfx}", [RROWS, 48], bf16, kind="ExternalInput").ap()
    d_who = nc.dram_tensor(f"who{sfx}", [KDIM, VOCAB], bf16, kind="ExternalInput").ap()
    d_out = nc.dram_tensor(f"out{sfx}", [PPC, VOCAB], f16, kind="ExternalOutput").ap()
    d_iter = (
        nc.dram_tensor(f"iters{sfx}", [1, 16], f32, kind="ExternalOutput").ap()
        if loop > 1
        else None
    )

    def body(tc):
        with tc.tile_pool(name="const", bufs=1) as cpool:
            R = cpool.tile([RROWS, POS], bf16)
            wall_s = cpool.tile([RROWS, 48], bf16)
            who_s = cpool.tile([KDIM, VOCAB], bf16)
            # Full-width stage (all 128 seq slots, not just this core's):
            # built with STATIC copies only -- dynamic (pid-based) slices of
            # partition-sliced tiles miscompile (stride-0 broadcast); the
            # only dynamic access is the matmul lhsT slice, which uses the
            # full-partition view form that lowers correctly.
            stage = cpool.tile([KDIM, POS], bf16)
            stage_c = cpool.tile([KDIM, PPC], bf16)
            # Tile 0's logits, staged to SBUF during the pipeline fill (DVE
            # is idle there); its pass 2 is then a cheap SBUF->SBUF add
            # with no second matmul pass.
            logits0 = cpool.tile([128, VOCAB], f16)

            # R initial content (h0 states + zero pad + all step inputs),
            # split into col chunks so step k only waits for its own chunk.
            nc.sync.dma_start(wall_s[:], d_wall[:])
            rb = [0, 128, 512] + [q * 512 for q in range(2, 9)]
            for q in range(len(rb) - 1):
                nc.sync.dma_start(
                    R[:, rb[q] : rb[q + 1]],
                    d_rstat[:, rb[q] : rb[q + 1]],
                )
            nc.sync.dma_start(who_s[:], d_who[:])

            # ---- Recurrence (replicated on every core) ----
            with tc.tile_pool(name="recpsum", bufs=4, space="PSUM") as rpsum:
                for k in range(SEQ - 1):
                    pk = rpsum.tile([48, 32], f32, tag="rp")
                    nc.tensor.matmul(
                        pk[:],
                        lhsT=wall_s[:],
                        rhs=R[:, k * 32 : (k + 1) * 32],
                        start=True,
                        stop=True,
                    )
                    nc.scalar.activation(
                        R[0:48, (k + 1) * 32 : (k + 2) * 32], pk[:], AF.Tanh
                    )

            # ---- Stage build (all 4096 positions, static copies) ----
            # stage col-block s: rows 0:16 = hLR[s], rows 32:48 = hRL[127-s],
            # row 64 = 1. Zero the pad rows first (uninitialised SBUF can
            # hold NaN bit patterns, and NaN * 0 = NaN in the logits matmul).
            pid = nc.partition_id()
            nc.vector.memset(stage[0:32, :], 0.0)
            nc.vector.memset(stage[32:64, :], 0.0)
            nc.vector.memset(stage[64:65, :], 1.0)
            # hLR in 4 chunks so early chunks copy while the recurrence
            # still runs.
            for q in range(4):
                nc.vector.tensor_copy(
                    stage[0:16, q * 1024 : (q + 1) * 1024],
                    R[0:16, q * 1024 : (q + 1) * 1024],
                )
            # hRL: reversed block mapping, one static copy per block,
            # emitted in the order their source blocks become ready.
            R3 = R[:].rearrange("p (k c) -> p k c", c=32)
            S3 = stage[:].rearrange("p (k c) -> p k c", c=32)
            for s in range(SEQ - 1, -1, -1):
                nc.vector.tensor_copy(
                    S3[32:48, s, :], R3[32:48, SEQ - 1 - s, :]
                )
            # This core's slice: ONE dynamic copy (full-partition view + ds
            # is the one dynamic form that lowers correctly). Matmul lhsT
            # requires static offsets, so it slices stage_c statically.
            nc.vector.tensor_copy(
                stage_c[:], stage[:][:, bass.ds(pid * PPC, PPC)]
            )

            # ---- Output stage ----
            with (
                tc.tile_pool(name="p1", bufs=2, space="PSUM") as p1pool,
                tc.tile_pool(name="p2", bufs=2, space="PSUM") as p2pool,
                tc.tile_pool(name="outst", bufs=7) as opool,
                tc.tile_pool(name="smalls", bufs=2) as smpool,
            ):
                negs = [None] * PTILES
                ots = [None] * PTILES

                def p1_chunk(t, sums, g, store=False):
                    c0, w = GROUPS[g]
                    lhsT = stage_c[:, t * 128 : (t + 1) * 128]
                    pg = p1pool.tile([128, GW], f32, tag="p1")
                    for j0, jw in _mm_splits(c0, w):
                        nc.tensor.matmul(
                            pg[:, j0 - c0 : j0 - c0 + jw],
                            lhsT=lhsT,
                            rhs=who_s[:, j0 : j0 + jw],
                            start=True,
                            stop=True,
                        )
                    if store:
                        # Second copy of the logits into the (otherwise
                        # idle) pass-2 PSUM pool via duplicate matmuls, so
                        # the DVE SBUF-staging copy and the in-place exp
                        # never touch the same PSUM tile. PE has big slack
                        # during the fill.
                        pq = p2pool.tile([128, GW], f32, tag="p2")
                        for j0, jw in _mm_splits(c0, w):
                            nc.tensor.matmul(
                                pq[:, j0 - c0 : j0 - c0 + jw],
                                lhsT=lhsT,
                                rhs=who_s[:, j0 : j0 + jw],
                                start=True,
                                stop=True,
                            )
                        nc.vector.tensor_copy(
                            logits0[:, c0 : c0 + w], pq[:, :w]
                        )
                    nc.scalar.activation(
                        pg[:, :w],
                        pg[:, :w],
                        AF.Exp,
                        accum_out=sums[:, g : g + 1],
                    )

                def reduce_ln(t, sums):
                    # neg = -ln(S) entirely on DVE so ACT never leaves the
                    # exp/tanh table set. S = m * 2^e, m in [1,2):
                    #   -ln(S) = -e*ln2 - ln(m), ln(m) via minimax poly.
                    S = smpool.tile([128, 1], f32, tag="S")
                    nc.vector.tensor_reduce(
                        S[:],
                        sums[:],
                        axis=mybir.AxisListType.X,
                        op=mybir.AluOpType.add,
                    )
                    i32 = mybir.dt.int32
                    bits = smpool.tile([128, 1], i32, tag="bits")
                    nc.vector.tensor_scalar(
                        bits[:],
                        S[:].bitcast(i32),
                        23,
                        None,
                        mybir.AluOpType.logical_shift_right,
                    )
                    nc.vector.tensor_scalar_add(bits[:], bits[:], -127)
                    e_f = smpool.tile([128, 1], f32, tag="e_f")
                    nc.vector.tensor_copy(e_f[:], bits[:])  # int -> float
                    mant = smpool.tile([128, 1], i32, tag="mant")
                    nc.vector.tensor_scalar(
                        mant[:],
                        S[:].bitcast(i32),
                        0x007FFFFF,
                        0x3F800000,
                        mybir.AluOpType.bitwise_and,
                        mybir.AluOpType.bitwise_or,
                    )
                    m = mant[:].bitcast(f32)
                    # ln(m) on [1,2): degree-5 poly in t = m - 1 for
                    # ln(1+t)/t, lstsq fit, |err on ln(m)| < 4e-6.
                    tt = smpool.tile([128, 1], f32, tag="tt")
                    nc.vector.tensor_scalar_add(tt[:], m, -1.0)
                    C = [0.99987663, -0.49760941, 0.31669577,
                         -0.19225670, 0.08450634, -0.01806849]
                    acc = smpool.tile([128, 1], f32, tag="acc")
                    nc.vector.tensor_scalar(
                        acc[:], tt[:], C[5], C[4],
                        mybir.AluOpType.mult, mybir.AluOpType.add,
                    )
                    for c in (C[3], C[2], C[1], C[0]):
                        nc.vector.tensor_tensor(
                            acc[:], acc[:], tt[:], mybir.AluOpType.mult
                        )
                        nc.vector.tensor_scalar_add(acc[:], acc[:], c)
                    # acc ~= ln(m)/t ; neg = -(e*ln2 + t*acc)
                    nc.vector.tensor_tensor(acc[:], acc[:], tt[:], mybir.AluOpType.mult)
                    neg = smpool.tile([128, 1], f32, tag="neg")
                    nc.vector.tensor_scalar(
                        neg[:], e_f[:], float(np.log(2.0)), None,
                        mybir.AluOpType.mult,
                    )
                    nc.vector.tensor_tensor(neg[:], neg[:], acc[:], mybir.AluOpType.add)
                    nc.vector.tensor_scalar_mul(neg[:], neg[:], -1.0)
                    negs[t] = neg

                def p2_chunk(t, g, tail=False):
                    # Chunk g of pass 2 for tile t: produce final values
                    # (+neg) in the stripe staging tile; DMA the stripe out
                    # when its last chunk lands. Tile 0 adds from its
                    # SBUF-staged logits (no matmul); others re-matmul and
                    # evacuate PSUM on DVE (or ACT, to balance load).
                    c0, w = GROUPS[g]
                    neg = negs[t]
                    si = g // 2
                    s0, sw = STRIPES[si]
                    if g % 2 == 0:
                        ot = opool.tile([128, SW], f16, tag="ot")
                        ots[t] = ot
                    ot = ots[t]
                    if t == 0:
                        nc.vector.tensor_scalar_add(
                            ot[:, c0 - s0 : c0 - s0 + w],
                            logits0[:, c0 : c0 + w],
                            neg[:, 0:1],
                        )
                    else:
                        lhsT = stage_c[:, t * 128 : (t + 1) * 128]
                        # Tail: whole stripes per engine (chunk-level
                        # alternation serialises on the shared stripe tile);
                        # ACT stripes draw PSUM from the now-idle pass-1
                        # pool so both engines have independent rotations.
                        on_act = (si % 2 == 1) if tail else (g % ACT_EVERY == 3)
                        if tail and on_act:
                            pq = p1pool.tile([128, GW], f32, tag="p1")
                        else:
                            pq = p2pool.tile([128, GW], f32, tag="p2")
                        for j0, jw in _mm_splits(c0, w):
                            nc.tensor.matmul(
                                pq[:, j0 - c0 : j0 - c0 + jw],
                                lhsT=lhsT,
                                rhs=who_s[:, j0 : j0 + jw],
                                start=True,
                                stop=True,
                            )
                        if on_act:
                            nc.scalar.activation(
                                ot[:, c0 - s0 : c0 - s0 + w],
                                pq[:, :w],
                                AF.Identity,
                                bias=neg[:, 0:1],
                            )
                        else:
                            nc.vector.tensor_scalar_add(
                                ot[:, c0 - s0 : c0 - s0 + w],
                                pq[:, :w],
                                neg[:, 0:1],
                            )
                    if g % 2 == 1 or g == len(GROUPS) - 1:
                        nc.sync.dma_start(
                            d_out[t * 128 : (t + 1) * 128, s0 : s0 + sw],
                            ot[:, :sw],
                        )

                NG = len(GROUPS)
                for t in range(PTILES):
                    sums = smpool.tile([128, NG], f32, tag="sums")
                    for g in range(NG):
                        p1_chunk(t, sums, g, store=(t == 0))
                        if t > 0:
                            p2_chunk(t - 1, g)
                    reduce_ln(t, sums)
                for g in range(NG):
                    p2_chunk(PTILES - 1, g, tail=True)

    with tile.TileContext(nc) as tc:
        if loop > 1:
            with tc.tile_pool(name="iterc", bufs=1) as ipool:
                acc = ipool.tile([1, 16], f32)
                nc.vector.memset(acc[:], 0.0)
                with tc.For_i(0, loop, 1):
                    nc.vector.tensor_scalar_add(acc[:], acc[:], 1.0)
                    body(tc)
                nc.sync.dma_start(d_iter[:], acc[:])
        else:
            body(tc)

    nc.compile()
    _CACHE[("nc", loop)] = nc
    return nc


def _prep(inputs):
    f32 = np.float32
    ids = np.asarray(inputs["input_batch"]).reshape(-1).astype(np.int64)
    emb = np.asarray(inputs["embedding"], dtype=f32)[ids]  # [4096, 32]
    emb3 = emb.reshape(SEQ, B, EMB)

    rstat = np.zeros((RROWS, POS), f32)
    rstat[0:16, 0:32] = np.asarray(inputs["h0_lr"], dtype=f32).T
    rstat[32:48, 0:32] = np.asarray(inputs["h0_rl"], dtype=f32).T
    # blk k: rows 48:80 = emb[k].T, rows 80:112 = emb[127-k].T, row 112 = 1
    rstat[48:80] = emb3.transpose(2, 0, 1).reshape(EMB, POS)
    rstat[80:112] = emb3[::-1].transpose(2, 0, 1).reshape(EMB, POS)
    rstat[112] = 1.0

    W_lr = np.asarray(inputs["W_lr"], dtype=f32)
    W_rl = np.asarray(inputs["W_rl"], dtype=f32)
    wall = np.zeros((RROWS, 48), f32)
    wall[0:16, 0:16] = W_lr[:, EMB:].T
    wall[48:80, 0:16] = W_lr[:, :EMB].T
    wall[112, 0:16] = np.asarray(inputs["b_lr"], dtype=f32)
    wall[32:48, 32:48] = W_rl[:, EMB:].T
    wall[80:112, 32:48] = W_rl[:, :EMB].T
    wall[112, 32:48] = np.asarray(inputs["b_rl"], dtype=f32)

    W_ho = np.asarray(inputs["W_ho"], dtype=f32)
    who = np.zeros((KDIM, VOCAB), f32)
    who[0:16] = W_ho[:, 0:16].T
    who[32:48] = W_ho[:, 16:32].T
    who[64] = np.asarray(inputs["b_ho"], dtype=f32)

    return {
        "rstat": rstat.astype(ml_dtypes.bfloat16),
        "wall": wall.astype(ml_dtypes.bfloat16),
        "who": who.astype(ml_dtypes.bfloat16),
    }


LAST_RESULTS = None


def kernel(**inputs):
    from concourse.bass_utils import run_bass_kernel_spmd

    nc = _build()
    in_map = _prep(inputs)
    trace = bool(int(os.environ.get("BASS_KERNEL_TRACE", "0")))
    res = run_bass_kernel_spmd(
        nc,
        [in_map] * NCORES,
        list(range(NCORES)),
        trace=trace,
    )
    global LAST_RESULTS
    LAST_RESULTS = res
    out = np.concatenate(
        [np.asarray(res.results[c]["out"]) for c in range(NCORES)], axis=0
    )
    return np.ascontiguousarray(
        out.reshape(SEQ, B, VOCAB).astype(np.float32)
    )


# revision 45
# speedup vs baseline: 3.2568x; 3.2568x over previous
"""Trainium2 Bass kernel for a bidirectional RNN language model.

Model: emb = embedding[input_batch]; two 16-wide tanh RNN scans (L->R and
R->L) over 128 steps; logits = [hLR, hRL_flipped] @ W_ho.T + b_ho;
log_softmax over vocab 32000. Output [128, 32, 32000] f32 (~524 MB).

Distribution: data-parallel over the 4096 flat (seq*batch) positions,
512 per core (core c owns seq slots [16c, 16c+16)). The tiny recurrence
is replicated on every core; each core computes logits + log_softmax for
its own positions only, selected at runtime via partition_id() dynamic
slices (no collectives -- softmax reduces over vocab, core-local).

Recurrence: ONE matmul + ONE tanh per step. The step state+input lives
in a single [113, 4096] bf16 SBUF tile R (one 32-col block per step):
  rows   0:16  hLR state   (written by tanh, partition base 0)
  rows  16:32  zeros       (pad so tanh output is one [48,32] op)
  rows  32:48  hRL state   (partition base 32)
  rows  48:80  emb[k]      (LR input, DMA-filled)
  rows  80:112 emb[127-k]  (RL input, DMA-filled)
  row   112    ones        (bias row, DMA-filled)
The merged weight Wall [113, 48] has zero blocks so a single
lhsT=Wall matmul computes both chains' next states into one [48,32]
PSUM tile; one tanh writes them back to R block k+1. Per-step latency
is sem/drain-bound (~660 ns in the cost model), so the recurrence is
~88 us regardless of matmul width; bf16 keeps DMA and copies cheap.

Output stage, two matmul passes per 128-position tile, interleaved at
1024-col chunk granularity across tiles (pass1 of tile t with pass2 of
tile t-1) so ACT / DVE / PE / DMA all run concurrently:
  pass1: logits matmul (lhsT = [65,128] bf16 stage slice) -> PSUM; ACT
  exp in-place with accum_out giving sum(exp) per chunk; -ln(sum) via a
  DVE bit-trick polynomial (no ACT table switch). Tile 0 additionally
  stages its logits to SBUF fp16 via DVE (idle during the fill), so its
  pass2 needs no matmul and runs as 2x-mode SBUF adds.
  pass2: matmul again + evacuate PSUM adding -ln(S) (DVE
  tensor_scalar_add; the post-pass1 tail splits whole output stripes
  between DVE and ACT Identity-with-bias, with separate PSUM pools per
  engine). DMA out in fp16 (halves the store-bandwidth floor; adds
  ~3e-4 rel err vs the 2e-2 gate). Host casts fp16->f32.

log_softmax max-subtraction is dropped: logits are bounded (~|7|), so
f32 exp cannot overflow; out = logits - ln(sum exp(logits)).

Host-side work is limited to layout transforms (transposes, zero-pad,
bias-row augmentation), the embedding row gather, and the final fp16->
f32 cast; all arithmetic (projections, recurrences, logits matmul,
softmax) runs on the NeuronCores.
"""

import os

import numpy as np
import ml_dtypes

SEQ, B, VOCAB = 128, 32, 32000
EMB, HID = 32, 16
NCORES = 8
POS = SEQ * B                 # 4096 flat positions, f = s*B + b
PPC = POS // NCORES           # 512 positions per core
PTILES = PPC // 128           # 4 position tiles of 128 per core
RROWS = 113                   # R tile rows (see module docstring)
KDIM = 65                     # stage/who contraction rows
GW = 1024                     # exp group / pass-2 chunk width (2 PSUM banks)
GROUPS = [(g * GW, GW) for g in range(31)] + [(31 * GW, VOCAB - 31 * GW)]
SW = 2048                     # output DMA stripe width (0.5 MB fp16)
STRIPES = [(s * SW, min(SW, VOCAB - s * SW)) for s in range((VOCAB + SW - 1) // SW)]
ACT_EVERY = 999               # 1 of every ACT_EVERY pass-2 chunks evacuates on ACT


def _mm_splits(j0, w):
    out = []
    j = j0
    while j < j0 + w:
        jw = min(512, j0 + w - j)
        out.append((j, jw))
        j += jw
    return out


_CACHE = {}


def _build(loop=1):
    if ("nc", loop) in _CACHE:
        return _CACHE[("nc", loop)]

    import concourse.bass as bass
    import concourse.tile as tile
    from concourse import bacc, mybir

    f32 = mybir.dt.float32
    f16 = mybir.dt.float16
    bf16 = mybir.dt.bfloat16
    AF = mybir.ActivationFunctionType

    nc = bacc.Bacc(
        "TRN2",
        target_bir_lowering=False,
        debug=False,
        num_devices=NCORES,
    )

    # Name suffix per loop variant: the jitted HLO is otherwise identical
    # across variants (the bass module travels out-of-band), so XLA's
    # compile cache would hand the loop build the non-loop NEFF.
    sfx = f"_L{loop}" if loop > 1 else ""
    d_rstat = nc.dram_tensor(f"rstat{sfx}", [RROWS, POS], bf16, kind="ExternalInput").ap()
    d_wall = nc.dram_tensor(f"wall{sfx}", [RROWS, 48], bf16, kind="ExternalInput").ap()
    d_who = nc.dram_tensor(f"who{sfx}", [KDIM, VOCAB], bf16, kind="ExternalInput").ap()
    d_out = nc.dram_tensor(f"out{sfx}", [PPC, VOCAB], f16, kind="ExternalOutput").ap()
    d_iter = (
        nc.dram_tensor(f"iters{sfx}", [1, 16], f32, kind="ExternalOutput").ap()
        if loop > 1
        else None
    )

    def body(tc):
        with tc.tile_pool(name="const", bufs=1) as cpool:
            R = cpool.tile([RROWS, POS], bf16)
            wall_s = cpool.tile([RROWS, 48], bf16)
            who_s = cpool.tile([KDIM, VOCAB], bf16)
            # Full-width stage (all 128 seq slots, not just this core's):
            # built with STATIC copies only -- dynamic (pid-based) slices of
            # partition-sliced tiles miscompile (stride-0 broadcast); the
            # only dynamic access is the matmul lhsT slice, which uses the
            # full-partition view form that lowers correctly.
            stage = cpool.tile([KDIM, POS], bf16)
            stage_c = cpool.tile([KDIM, PPC], bf16)
            # Tile 0's logits, staged to SBUF during the pipeline fill (DVE
            # is idle there); its pass 2 is then a cheap SBUF->SBUF add
            # with no second matmul pass.
            logits0 = cpool.tile([128, VOCAB], f16)

            # R initial content (h0 states + zero pad + all step inputs),
            # split into col chunks so step k only waits for its own chunk.
            nc.sync.dma_start(wall_s[:], d_wall[:])
            rb = [0, 128, 512] + [q * 512 for q in range(2, 9)]
            for q in range(len(rb) - 1):
                nc.sync.dma_start(
                    R[:, rb[q] : rb[q + 1]],
                    d_rstat[:, rb[q] : rb[q + 1]],
                )
            nc.sync.dma_start(who_s[:], d_who[:])

            # ---- Recurrence (replicated on every core) ----
            with tc.tile_pool(name="recpsum", bufs=4, space="PSUM") as rpsum:
                for k in range(SEQ - 1):
                    pk = rpsum.tile([48, 32], f32, tag="rp")
                    nc.tensor.matmul(
                        pk[:],
                        lhsT=wall_s[:],
                        rhs=R[:, k * 32 : (k + 1) * 32],
                        start=True,
                        stop=True,
                    )
                    nc.scalar.activation(
                        R[0:48, (k + 1) * 32 : (k + 2) * 32], pk[:], AF.Tanh
                    )

            # ---- Stage build (all 4096 positions, static copies) ----
            # stage col-block s: rows 0:16 = hLR[s], rows 32:48 = hRL[127-s],
            # row 64 = 1. Zero the pad rows first (uninitialised SBUF can
            # hold NaN bit patterns, and NaN * 0 = NaN in the logits matmul).
            pid = nc.partition_id()
            nc.vector.memset(stage[0:32, :], 0.0)
            nc.vector.memset(stage[32:64, :], 0.0)
            nc.vector.memset(stage[64:65, :], 1.0)
            # hLR in 4 chunks so early chunks copy while the recurrence
            # still runs.
            for q in range(4):
                nc.vector.tensor_copy(
                    stage[0:16, q * 1024 : (q + 1) * 1024],
                    R[0:16, q * 1024 : (q + 1) * 1024],
                )
            # hRL: reversed block mapping, one static copy per block,
            # emitted in the order their source blocks become ready.
            R3 = R[:].rearrange("p (k c) -> p k c", c=32)
            S3 = stage[:].rearrange("p (k c) -> p k c", c=32)
            for s in range(SEQ - 1, -1, -1):
                nc.vector.tensor_copy(
                    S3[32:48, s, :], R3[32:48, SEQ - 1 - s, :]
                )
            # This core's slice: ONE dynamic copy (full-partition view + ds
            # is the one dynamic form that lowers correctly). Matmul lhsT
            # requires static offsets, so it slices stage_c statically.
            nc.vector.tensor_copy(
                stage_c[:], stage[:][:, bass.ds(pid * PPC, PPC)]
            )

            # ---- Output stage ----
            with (
                tc.tile_pool(name="p1", bufs=2, space="PSUM") as p1pool,
                tc.tile_pool(name="p2", bufs=2, space="PSUM") as p2pool,
                tc.tile_pool(name="outst", bufs=7) as opool,
                tc.tile_pool(name="smalls", bufs=2) as smpool,
            ):
                negs = [None] * PTILES
                ots = [None] * PTILES

                def p1_chunk(t, sums, g, store=False):
                    c0, w = GROUPS[g]
                    lhsT = stage_c[:, t * 128 : (t + 1) * 128]
                    pg = p1pool.tile([128, GW], f32, tag="p1")
                    for j0, jw in _mm_splits(c0, w):
                        nc.tensor.matmul(
                            pg[:, j0 - c0 : j0 - c0 + jw],
                            lhsT=lhsT,
                            rhs=who_s[:, j0 : j0 + jw],
                            start=True,
                            stop=True,
                        )
                    if store:
                        # Second copy of the logits into the (otherwise
                        # idle) pass-2 PSUM pool via duplicate matmuls, so
                        # the DVE SBUF-staging copy and the in-place exp
                        # never touch the same PSUM tile. PE has big slack
                        # during the fill.
                        pq = p2pool.tile([128, GW], f32, tag="p2")
                        for j0, jw in _mm_splits(c0, w):
                            nc.tensor.matmul(
                                pq[:, j0 - c0 : j0 - c0 + jw],
                                lhsT=lhsT,
                                rhs=who_s[:, j0 : j0 + jw],
                                start=True,
                                stop=True,
                            )
                        nc.vector.tensor_copy(
                            logits0[:, c0 : c0 + w], pq[:, :w]
                        )
                    nc.scalar.activation(
                        pg[:, :w],
                        pg[:, :w],
                        AF.Exp,
                        accum_out=sums[:, g : g + 1],
                    )

                def reduce_ln(t, sums):
                    # neg = -ln(S) entirely on DVE so ACT never leaves the
                    # exp/tanh table set. S = m * 2^e, m in [1,2):
                    #   -ln(S) = -e*ln2 - ln(m), ln(m) via minimax poly.
                    S = smpool.tile([128, 1], f32, tag="S")
                    nc.vector.tensor_reduce(
                        S[:],
                        sums[:],
                        axis=mybir.AxisListType.X,
                        op=mybir.AluOpType.add,
                    )
                    i32 = mybir.dt.int32
                    bits = smpool.tile([128, 1], i32, tag="bits")
                    nc.vector.tensor_scalar(
                        bits[:],
                        S[:].bitcast(i32),
                        23,
                        None,
                        mybir.AluOpType.logical_shift_right,
                    )
                    nc.vector.tensor_scalar_add(bits[:], bits[:], -127)
                    e_f = smpool.tile([128, 1], f32, tag="e_f")
                    nc.vector.tensor_copy(e_f[:], bits[:])  # int -> float
                    mant = smpool.tile([128, 1], i32, tag="mant")
                    nc.vector.tensor_scalar(
                        mant[:],
                        S[:].bitcast(i32),
                        0x007FFFFF,
                        0x3F800000,
                        mybir.AluOpType.bitwise_and,
                        mybir.AluOpType.bitwise_or,
                    )
                    m = mant[:].bitcast(f32)
                    # ln(m) on [1,2): degree-5 poly in t = m - 1 for
                    # ln(1+t)/t, lstsq fit, |err on ln(m)| < 4e-6.
                    tt = smpool.tile([128, 1], f32, tag="tt")
                    nc.vector.tensor_scalar_add(tt[:], m, -1.0)
                    C = [0.99987663, -0.49760941, 0.31669577,
                         -0.19225670, 0.08450634, -0.01806849]
                    acc = smpool.tile([128, 1], f32, tag="acc")
                    nc.vector.tensor_scalar(
                        acc[:], tt[:], C[5], C[4],
                        mybir.AluOpType.mult, mybir.AluOpType.add,
                    )
                    for c in (C[3], C[2], C[1], C[0]):
                        nc.vector.tensor_tensor(
                            acc[:], acc[:], tt[:], mybir.AluOpType.mult
                        )
                        nc.vector.tensor_scalar_add(acc[:], acc[:], c)
                    # acc ~= ln(m)/t ; neg = -(e*ln2 + t*acc)
                    nc.vector.tensor_tensor(acc[:], acc[:], tt[:], mybir.AluOpType.mult)
                    neg = smpool.tile([128, 1], f32, tag="neg")
                    nc.vector.tensor_scalar(
                        neg[:], e_f[:], float(np.log(2.0)), None,
                        mybir.AluOpType.mult,
                    )
                    nc.vector.tensor_tensor(neg[:], neg[:], acc[:], mybir.AluOpType.add)
                    nc.vector.tensor_scalar_mul(neg[:], neg[:], -1.0)
                    negs[t] = neg

                def p2_chunk(t, g, tail=False):
                    # Chunk g of pass 2 for tile t: produce final values
                    # (+neg) in the stripe staging tile; DMA the stripe out
                    # when its last chunk lands. Tile 0 adds from its
                    # SBUF-staged logits (no matmul); others re-matmul and
                    # evacuate PSUM on DVE (or ACT, to balance load).
                    c0, w = GROUPS[g]
                    neg = negs[t]
                    si = g // 2
                    s0, sw = STRIPES[si]
                    if g % 2 == 0:
                        ot = opool.tile([128, SW], f16, tag="ot")
                        ots[t] = ot
                    ot = ots[t]
                    if t == 0:
                        nc.vector.tensor_scalar_add(
                            ot[:, c0 - s0 : c0 - s0 + w],
                            logits0[:, c0 : c0 + w],
                            neg[:, 0:1],
                        )
                    else:
                        lhsT = stage_c[:, t * 128 : (t + 1) * 128]
                        # Tail: alternate whole chunks between DVE and ACT;
                        # each chunk has its own stripe tile and the shared
                        # PSUM rotation alternates engines naturally.
                        on_act = (si % 2 == 1) if tail else (g % ACT_EVERY == 3)
                        if tail and on_act:
                            pq = p1pool.tile([128, GW], f32, tag="p1")
                        else:
                            pq = p2pool.tile([128, GW], f32, tag="p2")
                        for j0, jw in _mm_splits(c0, w):
                            nc.tensor.matmul(
                                pq[:, j0 - c0 : j0 - c0 + jw],
                                lhsT=lhsT,
                                rhs=who_s[:, j0 : j0 + jw],
                                start=True,
                                stop=True,
                            )
                        if on_act:
                            nc.scalar.activation(
                                ot[:, c0 - s0 : c0 - s0 + w],
                                pq[:, :w],
                                AF.Identity,
                                bias=neg[:, 0:1],
                            )
                        else:
                            nc.vector.tensor_scalar_add(
                                ot[:, c0 - s0 : c0 - s0 + w],
                                pq[:, :w],
                                neg[:, 0:1],
                            )
                    if g % 2 == 1 or g == len(GROUPS) - 1:
                        nc.sync.dma_start(
                            d_out[t * 128 : (t + 1) * 128, s0 : s0 + sw],
                            ot[:, :sw],
                        )

                NG = len(GROUPS)
                for t in range(PTILES):
                    sums = smpool.tile([128, NG], f32, tag="sums")
                    for g in range(NG):
                        p1_chunk(t, sums, g, store=(t == 0))
                        if t > 0:
                            p2_chunk(t - 1, g)
                    reduce_ln(t, sums)
                for g in range(NG):
                    p2_chunk(PTILES - 1, g, tail=True)

    with tile.TileContext(nc) as tc:
        if loop > 1:
            with tc.tile_pool(name="iterc", bufs=1) as ipool:
                acc = ipool.tile([1, 16], f32)
                nc.vector.memset(acc[:], 0.0)
                with tc.For_i(0, loop, 1):
                    nc.vector.tensor_scalar_add(acc[:], acc[:], 1.0)
                    body(tc)
                nc.sync.dma_start(d_iter[:], acc[:])
        else:
            body(tc)

    nc.compile()
    _CACHE[("nc", loop)] = nc
    return nc


def _prep(inputs):
    f32 = np.float32
    ids = np.asarray(inputs["input_batch"]).reshape(-1).astype(np.int64)
    emb = np.asarray(inputs["embedding"], dtype=f32)[ids]  # [4096, 32]
    emb3 = emb.reshape(SEQ, B, EMB)

    rstat = np.zeros((RROWS, POS), f32)
    rstat[0:16, 0:32] = np.asarray(inputs["h0_lr"], dtype=f32).T
    rstat[32:48, 0:32] = np.asarray(inputs["h0_rl"], dtype=f32).T
    # blk k: rows 48:80 = emb[k].T, rows 80:112 = emb[127-k].T, row 112 = 1
    rstat[48:80] = emb3.transpose(2, 0, 1).reshape(EMB, POS)
    rstat[80:112] = emb3[::-1].transpose(2, 0, 1).reshape(EMB, POS)
    rstat[112] = 1.0

    W_lr = np.asarray(inputs["W_lr"], dtype=f32)
    W_rl = np.asarray(inputs["W_rl"], dtype=f32)
    wall = np.zeros((RROWS, 48), f32)
    wall[0:16, 0:16] = W_lr[:, EMB:].T
    wall[48:80, 0:16] = W_lr[:, :EMB].T
    wall[112, 0:16] = np.asarray(inputs["b_lr"], dtype=f32)
    wall[32:48, 32:48] = W_rl[:, EMB:].T
    wall[80:112, 32:48] = W_rl[:, :EMB].T
    wall[112, 32:48] = np.asarray(inputs["b_rl"], dtype=f32)

    W_ho = np.asarray(inputs["W_ho"], dtype=f32)
    who = np.zeros((KDIM, VOCAB), f32)
    who[0:16] = W_ho[:, 0:16].T
    who[32:48] = W_ho[:, 16:32].T
    who[64] = np.asarray(inputs["b_ho"], dtype=f32)

    return {
        "rstat": rstat.astype(ml_dtypes.bfloat16),
        "wall": wall.astype(ml_dtypes.bfloat16),
        "who": who.astype(ml_dtypes.bfloat16),
    }


LAST_RESULTS = None


def kernel(**inputs):
    from concourse.bass_utils import run_bass_kernel_spmd

    nc = _build()
    in_map = _prep(inputs)
    trace = bool(int(os.environ.get("BASS_KERNEL_TRACE", "0")))
    res = run_bass_kernel_spmd(
        nc,
        [in_map] * NCORES,
        list(range(NCORES)),
        trace=trace,
    )
    global LAST_RESULTS
    LAST_RESULTS = res
    out = np.concatenate(
        [np.asarray(res.results[c]["out"]) for c in range(NCORES)], axis=0
    )
    return np.ascontiguousarray(
        out.reshape(SEQ, B, VOCAB).astype(np.float32)
    )


# revision 53
# speedup vs baseline: 6.4023x; 1.9658x over previous
"""Trainium2 Bass kernel for a bidirectional RNN language model.

Model: emb = embedding[input_batch]; two 16-wide tanh RNN scans (L->R and
R->L) over 128 steps; logits = [hLR, hRL_flipped] @ W_ho.T + b_ho;
log_softmax over vocab 32000. Output [128, 32, 32000] f32 (~524 MB).

Distribution: data-parallel over the 4096 flat (seq*batch) positions,
512 per core (core c owns seq slots [16c, 16c+16)). The tiny recurrence
is replicated on every core; each core computes logits + log_softmax for
its own positions only, selected at runtime via partition_id() dynamic
slices (no collectives -- softmax reduces over vocab, core-local).

Recurrence: ONE matmul + ONE tanh per step. The step state+input lives
in a single [113, 4096] bf16 SBUF tile R (one 32-col block per step):
  rows   0:16  hLR state   (written by tanh, partition base 0)
  rows  16:32  zeros       (pad so tanh output is one [48,32] op)
  rows  32:48  hRL state   (partition base 32)
  rows  48:80  emb[k]      (LR input, DMA-filled)
  rows  80:112 emb[127-k]  (RL input, DMA-filled)
  row   112    ones        (bias row, DMA-filled)
The merged weight Wall [113, 48] has zero blocks so a single
lhsT=Wall matmul computes both chains' next states into one [48,32]
PSUM tile; one tanh writes them back to R block k+1. Per-step latency
is sem/drain-bound (~660 ns in the cost model), so the recurrence is
~88 us regardless of matmul width; bf16 keeps DMA and copies cheap.

Output stage, two matmul passes per 128-position tile, interleaved at
1024-col chunk granularity across tiles (pass1 of tile t with pass2 of
tile t-1) so ACT / DVE / PE / DMA all run concurrently:
  pass1: logits matmul (lhsT = [65,128] bf16 stage slice) -> PSUM; ACT
  exp in-place with accum_out giving sum(exp) per chunk; -ln(sum) via a
  DVE bit-trick polynomial (no ACT table switch). Tile 0 additionally
  stages its logits to SBUF fp16 via DVE (idle during the fill), so its
  pass2 needs no matmul and runs as 2x-mode SBUF adds.
  pass2: matmul again + evacuate PSUM adding -ln(S) (DVE
  tensor_scalar_add; the post-pass1 tail splits whole output stripes
  between DVE and ACT Identity-with-bias, with separate PSUM pools per
  engine). DMA out in fp16 (halves the store-bandwidth floor; adds
  ~3e-4 rel err vs the 2e-2 gate). Host casts fp16->f32.

log_softmax max-subtraction is dropped: logits are bounded (~|7|), so
f32 exp cannot overflow; out = logits - ln(sum exp(logits)).

Host-side work is limited to layout transforms (transposes, zero-pad,
bias-row augmentation), the embedding row gather, and the final fp16->
f32 cast; all arithmetic (projections, recurrences, logits matmul,
softmax) runs on the NeuronCores.
"""

import os

import numpy as np
import ml_dtypes

SEQ, B, VOCAB = 128, 32, 32000
EMB, HID = 32, 16
NCORES = 8
POS = SEQ * B                 # 4096 flat positions, f = s*B + b
PPC = POS // NCORES           # 512 positions per core
PTILES = PPC // 128           # 4 position tiles of 128 per core
RROWS = 113                   # R tile rows (see module docstring)
KDIM = 65                     # stage/who contraction rows
GW1 = 1536                    # pass-1 exp group width (3 PSUM banks)
G1 = [(g * GW1, GW1) for g in range(20)] + [(20 * GW1, VOCAB - 20 * GW1)]
GW2 = 512                     # pass-2 evac chunk width (1 PSUM bank)
G2 = [(g * GW2, GW2) for g in range(62)] + [(62 * GW2, VOCAB - 62 * GW2)]
SW = 2048                     # output DMA stripe width (0.5 MB fp16)
STRIPES = [(s * SW, min(SW, VOCAB - s * SW)) for s in range((VOCAB + SW - 1) // SW)]
ACT_STRIPE_MID = 8            # middle: stripe si on ACT if si %% 8 == 3


def _mm_splits(j0, w):
    out = []
    j = j0
    while j < j0 + w:
        jw = min(512, j0 + w - j)
        out.append((j, jw))
        j += jw
    return out


_CACHE = {}


def _build(loop=1):
    if ("nc", loop) in _CACHE:
        return _CACHE[("nc", loop)]

    import concourse.bass as bass
    import concourse.tile as tile
    from concourse import bacc, mybir

    f32 = mybir.dt.float32
    f16 = mybir.dt.float16
    bf16 = mybir.dt.bfloat16
    AF = mybir.ActivationFunctionType

    nc = bacc.Bacc(
        "TRN2",
        target_bir_lowering=False,
        debug=False,
        num_devices=NCORES,
    )

    # Name suffix per loop variant: the jitted HLO is otherwise identical
    # across variants (the bass module travels out-of-band), so XLA's
    # compile cache would hand the loop build the non-loop NEFF.
    sfx = f"_L{loop}" if loop > 1 else ""
    d_rstat = nc.dram_tensor(f"rstat{sfx}", [RROWS, POS], bf16, kind="ExternalInput").ap()
    d_wall = nc.dram_tensor(f"wall{sfx}", [RROWS, 48], bf16, kind="ExternalInput").ap()
    d_who = nc.dram_tensor(f"who{sfx}", [KDIM, VOCAB], bf16, kind="ExternalInput").ap()
    d_out = nc.dram_tensor(f"out{sfx}", [PPC, VOCAB], f16, kind="ExternalOutput").ap()
    d_iter = (
        nc.dram_tensor(f"iters{sfx}", [1, 16], f32, kind="ExternalOutput").ap()
        if loop > 1
        else None
    )

    def body(tc):
        with tc.tile_pool(name="const", bufs=1) as cpool:
            R = cpool.tile([RROWS, POS], bf16)
            wall_s = cpool.tile([RROWS, 48], bf16)
            who_s = cpool.tile([KDIM, VOCAB], bf16)
            # Full-width stage (all 128 seq slots, not just this core's):
            # built with STATIC copies only -- dynamic (pid-based) slices of
            # partition-sliced tiles miscompile (stride-0 broadcast); the
            # only dynamic access is the matmul lhsT slice, which uses the
            # full-partition view form that lowers correctly.
            stage = cpool.tile([KDIM, POS], bf16)
            stage_c = cpool.tile([KDIM, PPC], bf16)
            # Tile 0's logits, staged to SBUF during the pipeline fill (DVE
            # is idle there); its pass 2 is then a cheap SBUF->SBUF add
            # with no second matmul pass.
            logits0 = cpool.tile([128, VOCAB], f16)

            # R initial content (h0 states + zero pad + all step inputs),
            # split into col chunks so step k only waits for its own chunk.
            nc.sync.dma_start(wall_s[:], d_wall[:])
            rb = [0, 128, 512] + [q * 512 for q in range(2, 9)]
            for q in range(len(rb) - 1):
                nc.sync.dma_start(
                    R[:, rb[q] : rb[q + 1]],
                    d_rstat[:, rb[q] : rb[q + 1]],
                )
            nc.sync.dma_start(who_s[:], d_who[:])

            # ---- Recurrence (replicated on every core) ----
            with tc.tile_pool(name="recpsum", bufs=4, space="PSUM") as rpsum:
                for k in range(SEQ - 1):
                    pk = rpsum.tile([48, 32], f32, tag="rp")
                    nc.tensor.matmul(
                        pk[:],
                        lhsT=wall_s[:],
                        rhs=R[:, k * 32 : (k + 1) * 32],
                        start=True,
                        stop=True,
                    )
                    nc.scalar.activation(
                        R[0:48, (k + 1) * 32 : (k + 2) * 32], pk[:], AF.Tanh
                    )

            # ---- Stage build (all 4096 positions, static copies) ----
            # stage col-block s: rows 0:16 = hLR[s], rows 32:48 = hRL[127-s],
            # row 64 = 1. Zero the pad rows first (uninitialised SBUF can
            # hold NaN bit patterns, and NaN * 0 = NaN in the logits matmul).
            pid = nc.partition_id()
            nc.vector.memset(stage[0:32, :], 0.0)
            nc.vector.memset(stage[32:64, :], 0.0)
            nc.vector.memset(stage[64:65, :], 1.0)
            # hLR in 4 chunks so early chunks copy while the recurrence
            # still runs.
            for q in range(4):
                nc.vector.tensor_copy(
                    stage[0:16, q * 1024 : (q + 1) * 1024],
                    R[0:16, q * 1024 : (q + 1) * 1024],
                )
            # hRL: reversed block mapping, one static copy per block,
            # emitted in the order their source blocks become ready.
            R3 = R[:].rearrange("p (k c) -> p k c", c=32)
            S3 = stage[:].rearrange("p (k c) -> p k c", c=32)
            for s in range(SEQ - 1, -1, -1):
                nc.vector.tensor_copy(
                    S3[32:48, s, :], R3[32:48, SEQ - 1 - s, :]
                )
            # This core's slice: ONE dynamic copy (full-partition view + ds
            # is the one dynamic form that lowers correctly). Matmul lhsT
            # requires static offsets, so it slices stage_c statically.
            nc.vector.tensor_copy(
                stage_c[:], stage[:][:, bass.ds(pid * PPC, PPC)]
            )

            # ---- Output stage ----
            with (
                tc.tile_pool(name="p1", bufs=2, space="PSUM") as p1pool,
                tc.tile_pool(name="p2", bufs=2, space="PSUM") as p2pool,
                tc.tile_pool(name="outst", bufs=7) as opool,
                tc.tile_pool(name="smalls", bufs=2) as smpool,
            ):
                negs = [None] * PTILES
                ots = [None] * PTILES

                def p1_chunk(t, sums, g):
                    c0, w = G1[g]
                    lhsT = stage_c[:, t * 128 : (t + 1) * 128]
                    pg = p1pool.tile([128, GW1], f32, tag="p1")
                    for j0, jw in _mm_splits(c0, w):
                        nc.tensor.matmul(
                            pg[:, j0 - c0 : j0 - c0 + jw],
                            lhsT=lhsT,
                            rhs=who_s[:, j0 : j0 + jw],
                            start=True,
                            stop=True,
                        )
                    nc.scalar.activation(
                        pg[:, :w],
                        pg[:, :w],
                        AF.Exp,
                        accum_out=sums[:, g : g + 1],
                    )

                def store_chunk(g):
                    # Tile 0 fill: logits copy to SBUF via duplicate matmuls
                    # into the pass-2 pool (PE has big slack during the
                    # fill; keeps the DVE copy off the exp PSUM tiles).
                    c0, w = G2[g]
                    lhsT = stage_c[:, 0:128]
                    pq = p2pool.tile([128, GW2], f32, tag="p2")
                    for j0, jw in _mm_splits(c0, w):
                        nc.tensor.matmul(
                            pq[:, j0 - c0 : j0 - c0 + jw],
                            lhsT=lhsT,
                            rhs=who_s[:, j0 : j0 + jw],
                            start=True,
                            stop=True,
                        )
                    nc.vector.tensor_copy(logits0[:, c0 : c0 + w], pq[:, :w])

                def reduce_ln(t, sums):
                    # neg = -ln(S) entirely on DVE so ACT never leaves the
                    # exp/tanh table set. S = m * 2^e, m in [1,2):
                    #   -ln(S) = -e*ln2 - ln(m), ln(m) via minimax poly.
                    S = smpool.tile([128, 1], f32, tag="S")
                    nc.vector.tensor_reduce(
                        S[:],
                        sums[:],
                        axis=mybir.AxisListType.X,
                        op=mybir.AluOpType.add,
                    )
                    i32 = mybir.dt.int32
                    bits = smpool.tile([128, 1], i32, tag="bits")
                    nc.vector.tensor_scalar(
                        bits[:],
                        S[:].bitcast(i32),
                        23,
                        None,
                        mybir.AluOpType.logical_shift_right,
                    )
                    nc.vector.tensor_scalar_add(bits[:], bits[:], -127)
                    e_f = smpool.tile([128, 1], f32, tag="e_f")
                    nc.vector.tensor_copy(e_f[:], bits[:])  # int -> float
                    mant = smpool.tile([128, 1], i32, tag="mant")
                    nc.vector.tensor_scalar(
                        mant[:],
                        S[:].bitcast(i32),
                        0x007FFFFF,
                        0x3F800000,
                        mybir.AluOpType.bitwise_and,
                        mybir.AluOpType.bitwise_or,
                    )
                    m = mant[:].bitcast(f32)
                    # ln(m) on [1,2): degree-5 poly in t = m - 1 for
                    # ln(1+t)/t, lstsq fit, |err on ln(m)| < 4e-6.
                    tt = smpool.tile([128, 1], f32, tag="tt")
                    nc.vector.tensor_scalar_add(tt[:], m, -1.0)
                    C = [0.99987663, -0.49760941, 0.31669577,
                         -0.19225670, 0.08450634, -0.01806849]
                    acc = smpool.tile([128, 1], f32, tag="acc")
                    nc.vector.tensor_scalar(
                        acc[:], tt[:], C[5], C[4],
                        mybir.AluOpType.mult, mybir.AluOpType.add,
                    )
                    for c in (C[3], C[2], C[1], C[0]):
                        nc.vector.tensor_tensor(
                            acc[:], acc[:], tt[:], mybir.AluOpType.mult
                        )
                        nc.vector.tensor_scalar_add(acc[:], acc[:], c)
                    # acc ~= ln(m)/t ; neg = -(e*ln2 + t*acc)
                    nc.vector.tensor_tensor(acc[:], acc[:], tt[:], mybir.AluOpType.mult)
                    neg = smpool.tile([128, 1], f32, tag="neg")
                    nc.vector.tensor_scalar(
                        neg[:], e_f[:], float(np.log(2.0)), None,
                        mybir.AluOpType.mult,
                    )
                    nc.vector.tensor_tensor(neg[:], neg[:], acc[:], mybir.AluOpType.add)
                    nc.vector.tensor_scalar_mul(neg[:], neg[:], -1.0)
                    negs[t] = neg

                def p2_chunk(t, g, tail=False):
                    # Chunk g of pass 2 for tile t: produce final values
                    # (+neg) in the stripe staging tile; DMA the stripe out
                    # when its last chunk lands. Tile 0 adds from its
                    # SBUF-staged logits (no matmul); others re-matmul and
                    # evacuate PSUM on DVE (or ACT for whole stripes, to
                    # balance engine load -- chunk-level engine mixing
                    # within one stripe tile serialises on its writes).
                    c0, w = G2[g]
                    neg = negs[t]
                    si = c0 // SW
                    s0, sw = STRIPES[si]
                    if g % 4 == 0:
                        ot = opool.tile([128, SW], f16, tag="ot")
                        ots[t] = ot
                    ot = ots[t]
                    if t == 0:
                        nc.vector.tensor_scalar_add(
                            ot[:, c0 - s0 : c0 - s0 + w],
                            logits0[:, c0 : c0 + w],
                            neg[:, 0:1],
                        )
                    else:
                        lhsT = stage_c[:, t * 128 : (t + 1) * 128]
                        on_act = (
                            (si % 2 == 1) if tail
                            else (si % ACT_STRIPE_MID == 3)
                        )
                        if on_act:
                            # ACT-evac tiles ride the p1 rotation: ACT's
                            # FIFO already orders them after the exps, so
                            # no extra stall; keeps DVE's p2 rotation clean.
                            pq = p1pool.tile([128, GW2], f32, tag="p1")
                        else:
                            pq = p2pool.tile([128, GW2], f32, tag="p2")
                        for j0, jw in _mm_splits(c0, w):
                            nc.tensor.matmul(
                                pq[:, j0 - c0 : j0 - c0 + jw],
                                lhsT=lhsT,
                                rhs=who_s[:, j0 : j0 + jw],
                                start=True,
                                stop=True,
                            )
                        if on_act:
                            nc.scalar.activation(
                                ot[:, c0 - s0 : c0 - s0 + w],
                                pq[:, :w],
                                AF.Identity,
                                bias=neg[:, 0:1],
                            )
                        else:
                            nc.vector.tensor_scalar_add(
                                ot[:, c0 - s0 : c0 - s0 + w],
                                pq[:, :w],
                                neg[:, 0:1],
                            )
                    if g % 4 == 3 or g == len(G2) - 1:
                        nc.sync.dma_start(
                            d_out[t * 128 : (t + 1) * 128, s0 : s0 + sw],
                            ot[:, :sw],
                        )

                N1, N2 = len(G1), len(G2)
                for t in range(PTILES):
                    sums = smpool.tile([128, N1], f32, tag="sums")
                    for i in range(N2):
                        if i % 3 == 0 and i // 3 < N1:
                            p1_chunk(t, sums, i // 3)
                        if t == 0:
                            store_chunk(i)
                        else:
                            p2_chunk(t - 1, i)
                    reduce_ln(t, sums)
                for g in range(N2):
                    p2_chunk(PTILES - 1, g, tail=True)

    with tile.TileContext(nc) as tc:
        if loop > 1:
            with tc.tile_pool(name="iterc", bufs=1) as ipool:
                acc = ipool.tile([1, 16], f32)
                nc.vector.memset(acc[:], 0.0)
                with tc.For_i(0, loop, 1):
                    nc.vector.tensor_scalar_add(acc[:], acc[:], 1.0)
                    body(tc)
                nc.sync.dma_start(d_iter[:], acc[:])
        else:
            body(tc)

    nc.compile()
    _CACHE[("nc", loop)] = nc
    return nc


def _prep(inputs):
    f32 = np.float32
    ids = np.asarray(inputs["input_batch"]).reshape(-1).astype(np.int64)
    emb = np.asarray(inputs["embedding"], dtype=f32)[ids]  # [4096, 32]
    emb3 = emb.reshape(SEQ, B, EMB)

    rstat = np.zeros((RROWS, POS), f32)
    rstat[0:16, 0:32] = np.asarray(inputs["h0_lr"], dtype=f32).T
    rstat[32:48, 0:32] = np.asarray(inputs["h0_rl"], dtype=f32).T
    # blk k: rows 48:80 = emb[k].T, rows 80:112 = emb[127-k].T, row 112 = 1
    rstat[48:80] = emb3.transpose(2, 0, 1).reshape(EMB, POS)
    rstat[80:112] = emb3[::-1].transpose(2, 0, 1).reshape(EMB, POS)
    rstat[112] = 1.0

    W_lr = np.asarray(inputs["W_lr"], dtype=f32)
    W_rl = np.asarray(inputs["W_rl"], dtype=f32)
    wall = np.zeros((RROWS, 48), f32)
    wall[0:16, 0:16] = W_lr[:, EMB:].T
    wall[48:80, 0:16] = W_lr[:, :EMB].T
    wall[112, 0:16] = np.asarray(inputs["b_lr"], dtype=f32)
    wall[32:48, 32:48] = W_rl[:, EMB:].T
    wall[80:112, 32:48] = W_rl[:, :EMB].T
    wall[112, 32:48] = np.asarray(inputs["b_rl"], dtype=f32)

    W_ho = np.asarray(inputs["W_ho"], dtype=f32)
    who = np.zeros((KDIM, VOCAB), f32)
    who[0:16] = W_ho[:, 0:16].T
    who[32:48] = W_ho[:, 16:32].T
    who[64] = np.asarray(inputs["b_ho"], dtype=f32)

    return {
        "rstat": rstat.astype(ml_dtypes.bfloat16),
        "wall": wall.astype(ml_dtypes.bfloat16),
        "who": who.astype(ml_dtypes.bfloat16),
    }


LAST_RESULTS = None


def kernel(**inputs):
    from concourse.bass_utils import run_bass_kernel_spmd

    nc = _build()
    in_map = _prep(inputs)
    trace = bool(int(os.environ.get("BASS_KERNEL_TRACE", "0")))
    res = run_bass_kernel_spmd(
        nc,
        [in_map] * NCORES,
        list(range(NCORES)),
        trace=trace,
    )
    global LAST_RESULTS
    LAST_RESULTS = res
    out = np.concatenate(
        [np.asarray(res.results[c]["out"]) for c in range(NCORES)], axis=0
    )
    return np.ascontiguousarray(
        out.reshape(SEQ, B, VOCAB).astype(np.float32)
    )
